# revision 1
# baseline (speedup 1.0000x reference)
"""DAG-BiNN exact-degree message passing on 8 TRN2 NeuronCores.

Graph: 20000 gene nodes -> 7 levels x 10000 nodes, in-degree 16 per dst.
Per step k: h_dst = tanh(sum_d h_prev[src]*w + bias); head: roots @ W + b.

Sharding: each core owns 1250 dst nodes (padded to 1280) of EVERY level and
the full batch (256). Levels live node-major [rows, 256] bf16 in DRAM; the
per-step gather is a dma_gather(transpose=True) producing batch-major
[128, 2, n_idx] bf16 tiles (split into <=896-idx sub-gathers: the ucode
fails between 896 and 1024 idxs/instr); DVE multiplies by edge weights
(partition-broadcast via DMA) and strided-reduces groups of 16; ACT applies
tanh; PE transposes back to node-major; an AllGather replicates each level
(levels 1..6). Level 7 stays local: each core computes a partial head
matmul [2, 256] and the host sums partials and adds head_b. node_bias adds
are emitted only when node_bias is nonzero (it is all-zeros here).
"""

import os

import numpy as np
import ml_dtypes

# ---- problem constants (hardcoded; kernel.py must be self-contained) ----
GENES = 20000
LEVEL = 10000
STEPS = 7
DEG = 16
B = 256
C = 2
NCORES = 8

PC_D = LEVEL // NCORES          # 1250 real dsts per core per level
CH_DST = 256                    # dsts per chunk
CH = 5                          # chunks per core per level
PC_DP = CH * CH_DST             # 1280 padded dsts per core
SLOT = DEG                      # 16 edge slots per dst
NIDX = CH_DST * SLOT            # 4096 gather indices per chunk
IDXCOL = NIDX // 16             # 256 idx columns (16-partition wrap)
LVL_ROWS = NCORES * PC_DP       # 10240 rows per level table
BHALF = B // 128                # 2 batch halves
SUB_MAX = 896                   # max idxs per dma_gather instruction

BF16 = ml_dtypes.bfloat16

_COMPILED = {}
LAST_RESULT = None  # BassKernelResults of the most recent run (for test.py)


def _lvlrow(pos):
    """Level-k position (0..9999) -> row in the padded level table."""
    return PC_DP * (pos // PC_D) + pos % PC_D


def _prep(inputs):
    """Host-side: build per-core index/weight tables from runtime inputs."""
    X = np.asarray(inputs["X"], np.float32)
    ew = np.asarray(inputs["edge_weight"], np.float32)
    nb = np.asarray(inputs["node_bias"], np.float32)
    hW = np.asarray(inputs["head_W"], np.float32)
    hb = np.asarray(inputs["head_b"], np.float32)
    gm = np.asarray(inputs["gene_map"]).astype(np.int64)
    src = np.asarray(inputs["src"]).astype(np.int64)
    dpos = np.asarray(inputs["dst_pos"]).astype(np.int64)
    du = np.asarray(inputs["dst_unique"]).astype(np.int64)
    eid = np.asarray(inputs["eid"]).astype(np.int64)
    roots = np.asarray(inputs["root_ids"]).astype(np.int64)

    assert X.shape == (B, GENES) and src.shape == (STEPS, LEVEL * DEG)

    # X table: row g holds node gene_map[g]'s batch vector.
    x_tab = np.ascontiguousarray(X.T.astype(BF16))
    row_of_gene = np.empty(GENES, np.int64)
    row_of_gene[gm] = np.arange(GENES)

    max_id = int(du.max()) + 1
    idx_all = np.empty((NCORES, STEPS, CH, 16, IDXCOL), np.int16)
    w_all = np.empty((NCORES, STEPS * CH, NIDX), BF16)
    bias_all = np.zeros((NCORES, STEPS * CH, CH_DST), np.float32)

    lp = np.arange(PC_DP)
    valid = lp < PC_D

    for k in range(STEPS):
        order = np.argsort(dpos[k], kind="stable")
        assert np.all(np.bincount(dpos[k], minlength=LEVEL) == DEG), (
            "kernel assumes exact in-degree 16"
        )
        srcs = src[k][order]                 # [LEVEL*DEG] sorted by dst
        ws = ew[eid[k][order]]

        if k == 0:
            rows = row_of_gene[srcs]
        else:
            pos_of = np.full(max_id, -1, np.int64)
            pos_of[du[k - 1]] = np.arange(LEVEL)
            p = pos_of[srcs]
            assert (p >= 0).all(), "src outside previous level"
            rows = _lvlrow(p)

        rows = rows.reshape(LEVEL, DEG)
        ws = ws.reshape(LEVEL, DEG)
        for c in range(NCORES):
            pos_v = c * PC_D + lp[valid]  # real level positions for this core
            slot_idx = np.zeros((PC_DP, SLOT), np.int64)
            slot_w = np.zeros((PC_DP, SLOT), np.float32)
            slot_idx[valid] = rows[pos_v]
            slot_w[valid] = ws[pos_v]
            bias_all[c, k * CH:(k + 1) * CH] = np.where(
                valid, nb[du[k][np.minimum(lp + c * PC_D, LEVEL - 1)]], 0.0
            ).reshape(CH, CH_DST)
            # chunk + wrap: idx i=(jl2*16+d) -> [i%16, i//16]
            fl = slot_idx.reshape(CH, NIDX).astype(np.int16)
            idx_all[c, k] = fl.reshape(CH, IDXCOL, 16).transpose(0, 2, 1)
            w_all[c, k * CH:(k + 1) * CH] = slot_w.reshape(CH, NIDX).astype(BF16)

    # idx_tab per core: [128, STEPS*CH*IDXCOL], 16-row pattern tiled to 128
    idx16 = idx_all.reshape(NCORES, STEPS * CH, 16, IDXCOL)
    idx16 = idx16.transpose(0, 2, 1, 3).reshape(NCORES, 16, STEPS * CH * IDXCOL)
    idx_tab = np.tile(idx16, (1, NCORES, 1))  # [NCORES, 128, S*CH*IDXCOL]

    # head: W_eff[node] = sum of head_W rows whose root_ids hit that node
    W_eff = np.zeros((max_id, C), np.float32)
    np.add.at(W_eff, np.minimum(roots, max_id - 1), hW)
    head_tabs = []
    for c in range(NCORES):
        Wc = np.zeros((PC_DP, C), np.float32)
        Wc[valid] = W_eff[du[STEPS - 1][c * PC_D + lp[valid]]]
        head_tabs.append(
            np.ascontiguousarray(
                Wc.reshape(CH * BHALF, 128, C).transpose(1, 0, 2)
            ).astype(BF16)
        )  # [128, 10, 2]

    has_bias = bool(np.any(nb != 0.0))
    in_maps = []
    for c in range(NCORES):
        m = {
            "x_tab": x_tab,
            "idx_tab": np.ascontiguousarray(idx_tab[c]),
            "w_tab": np.ascontiguousarray(w_all[c]),
            "head_w": head_tabs[c],
        }
        if has_bias:
            m["bias_tab"] = np.ascontiguousarray(bias_all[c])
        in_maps.append(m)
    return in_maps, hb, has_bias


def _build_nc(has_bias):
    import concourse.bacc as bacc
    import concourse.mybir as mybir
    import concourse.tile as tile
    from concourse.masks import make_identity

    f32 = mybir.dt.float32
    bf16 = mybir.dt.bfloat16
    i16 = mybir.dt.int16

    nc = bacc.Bacc(num_devices=NCORES)
    x_tab = nc.declare_dram_parameter("x_tab", [GENES, B], bf16, isOutput=False)
    idx_tab = nc.declare_dram_parameter(
        "idx_tab", [128, STEPS * CH * IDXCOL], i16, isOutput=False
    )
    w_tab = nc.declare_dram_parameter(
        "w_tab", [STEPS * CH, NIDX], bf16, isOutput=False
    )
    head_w = nc.declare_dram_parameter(
        "head_w", [128, CH * BHALF, C], bf16, isOutput=False
    )
    bias_tab = None
    if has_bias:
        bias_tab = nc.declare_dram_parameter(
            "bias_tab", [STEPS * CH, CH_DST], f32, isOutput=False
        )
    out_partial = nc.declare_dram_parameter("out_partial", [C, B], f32, isOutput=True)

    # level tables (gather sources); collective output should be Shared
    lvl = [
        nc.dram_tensor(f"lvl{i}", [LVL_ROWS, B], bf16, addr_space="Shared")
        for i in range(2)
    ]
    own_slice = nc.dram_tensor("own_slice", [PC_DP, B], bf16)

    # sub-gather split: <=896 idxs per instruction, multiples of 128
    subs = []
    off = 0
    while off < NIDX:
        ni = min(SUB_MAX, NIDX - off)
        subs.append((off, ni))
        off += ni

    with tile.TileContext(nc) as tc:
        with (
            tc.tile_pool(name="const", bufs=1) as const_pool,
            tc.tile_pool(name="msg", bufs=3) as msg_pool,
            tc.tile_pool(name="wsb", bufs=2) as w_pool,
            tc.tile_pool(name="agg", bufs=2) as agg_pool,
            tc.tile_pool(name="hch", bufs=2) as h_pool,
            tc.tile_pool(name="ps", bufs=4, space="PSUM") as psum_pool,
            tc.tile_pool(name="pshead", bufs=1, space="PSUM") as psum_head,
        ):
            # --- persistent setup ---
            idx_sb = const_pool.tile([128, STEPS * CH * IDXCOL], i16)
            nc.sync.dma_start(out=idx_sb[:], in_=idx_tab[:, :])
            hw_sb = const_pool.tile([128, CH * BHALF, C], bf16)
            nc.sync.dma_start(out=hw_sb[:], in_=head_w[:, :, :])
            ident = const_pool.tile([128, 128], bf16)
            make_identity(nc, ident[:])
            nm_sb = const_pool.tile([128, CH * BHALF, B], bf16)
            sub_regs = {ni: nc.gpsimd.to_reg(ni) for ni in {s[1] for s in subs}}

            for k in range(STEPS):
                srctab = x_tab if k == 0 else lvl[(k - 1) % 2]
                for u in range(CH):
                    r = k * CH + u
                    w_sb = w_pool.tile([128, NIDX], bf16)
                    nc.sync.dma_start(
                        out=w_sb[:], in_=w_tab[r : r + 1, :].to_broadcast([128, NIDX])
                    )
                    agg = agg_pool.tile([128, BHALF, CH_DST], f32)
                    for i0, ni in subs:
                        # exact-size tile: gather out must be contiguous
                        msg = msg_pool.tile([128, BHALF, ni], bf16, tag="msg")
                        nc.gpsimd.dma_gather(
                            out_ap=msg[:],
                            in_ap=srctab[:, :],
                            idxs_ap=idx_sb[
                                :,
                                r * IDXCOL + i0 // 16 : r * IDXCOL + (i0 + ni) // 16,
                            ],
                            num_idxs=ni,
                            num_idxs_reg=sub_regs[ni],
                            elem_size=B,
                            transpose=True,
                        )
                        for jb in range(BHALF):
                            nc.vector.tensor_tensor(
                                out=msg[:, jb, :],
                                in0=msg[:, jb, :],
                                in1=w_sb[:, i0 : i0 + ni],
                                op=mybir.AluOpType.mult,
                            )
                        nd = ni // SLOT
                        j0 = i0 // SLOT
                        nc.vector.reduce_sum(
                            out=agg[:, :, j0 : j0 + nd],
                            in_=msg[:].rearrange("p a (j d) -> p a j d", d=SLOT),
                            axis=mybir.AxisListType.X,
                        )
                    if has_bias:
                        bias_bc = w_pool.tile([128, CH_DST], f32, tag="biasbc")
                        nc.sync.dma_start(
                            out=bias_bc[:],
                            in_=bias_tab[r : r + 1, :].to_broadcast([128, CH_DST]),
                        )
                        for jb in range(BHALF):
                            nc.vector.tensor_tensor(
                                out=agg[:, jb, :],
                                in0=agg[:, jb, :],
                                in1=bias_bc[:],
                                op=mybir.AluOpType.add,
                            )
                    hch = h_pool.tile([128, BHALF, CH_DST], bf16)
                    nc.scalar.activation(
                        out=hch[:], in_=agg[:], func=mybir.ActivationFunctionType.Tanh
                    )
                    # transpose [batch, dst] -> [dst, batch] via PE
                    for jb in range(BHALF):
                        for t2 in range(CH_DST // 128):
                            pt = psum_pool.tile([128, 128], bf16)
                            nc.tensor.transpose(
                                out=pt[:],
                                in_=hch[:, jb, t2 * 128 : (t2 + 1) * 128],
                                identity=ident[:],
                            )
                            nc.scalar.copy(
                                out=nm_sb[
                                    :, u * 2 + t2, jb * 128 : (jb + 1) * 128
                                ],
                                in_=pt[:],
                            )
                if k < STEPS - 1:
                    # publish level k+1: own 1280 rows -> all-gather to lvl[k%2]
                    # own_slice row (t*128+p) <- nm_sb[p, t, :]
                    nc.sync.dma_start(
                        out=own_slice[:, :].rearrange("(t p) b -> p t b", p=128),
                        in_=nm_sb[:],
                    )
                    nc.gpsimd.collective_compute(
                        "AllGather",
                        mybir.AluOpType.bypass,
                        replica_groups=[list(range(NCORES))],
                        ins=[own_slice[:, :]],
                        outs=[lvl[k % 2][:, :]],
                    )
                else:
                    pm = psum_head.tile([C, B], f32)
                    nt = CH * BHALF
                    for t in range(nt):
                        nc.tensor.matmul(
                            out=pm[:],
                            lhsT=hw_sb[:, t, :],
                            rhs=nm_sb[:, t, :],
                            start=(t == 0),
                            stop=(t == nt - 1),
                        )
                    res = const_pool.tile([C, B], f32)
                    nc.vector.tensor_copy(out=res[:], in_=pm[:])
                    nc.sync.dma_start(out=out_partial[:, :], in_=res[:])
    nc.finalize()
    return nc


def kernel(**inputs):
    global LAST_RESULT
    from concourse.bass_utils import run_bass_kernel_spmd

    in_maps, hb, has_bias = _prep(inputs)

    key = ("nc", has_bias)
    if key not in _COMPILED:
        _COMPILED[key] = _build_nc(has_bias)
    nc = _COMPILED[key]

    trace = os.environ.get("BASS_TRACE", "0") == "1"
    res = run_bass_kernel_spmd(
        nc, in_maps, core_ids=list(range(NCORES)), trace=trace
    )
    LAST_RESULT = res

    partials = np.stack(
        [np.asarray(r["out_partial"], np.float32) for r in res.results]
    )
    out = partials.sum(axis=0).T + hb[None, :]
    return out.astype(np.float32)



# revision 2
# speedup vs baseline: 1.1136x; 1.1136x over previous
"""DAG-BiNN exact-degree message passing on 8 TRN2 NeuronCores — v2.

Graph: 20000 gene nodes -> 7 levels x 10000 nodes, in-degree 16 per dst.
Per step k: h_dst = tanh(sum_d h_prev[src]*w + bias); head: roots @ W + b.

v2 design (vs v1): node-major everywhere.
- dma_gather(transpose=False) pulls src rows [256 batch bf16 = 512B] from the
  DRAM level table straight into SBUF as [128, nblk, 256]: gathered row i ->
  partition i%128, block i//128. Rows are in dst-major edge order, so block c
  holds 8 dsts x 16 slots.
- The multiply-by-edge-weight AND the 16-way reduction happen on the PE:
  per 64-dst group, 8 PSUM-accumulated matmuls with lhsT = host-built
  [128, 64] weight block (row k of block b -> column 8b + k//16 carries
  w(dst, slot); the rest zeros), rhs = gathered block [128, 256], out =
  psum[64g:64g+64, :] node-major (matmul PSUM base must be 0/32/64).
  Two groups fill a [128, 256] psum tile = 128 dsts. No DVE work, no
  weight broadcasts, no PE transposes.
- ACT applies tanh PSUM -> nm_sb (node-major bf16), optionally adding
  node_bias via the per-partition bias operand.
- Publish: own slice DMA to DRAM + AllGather (levels 1..6); level 7 local
  head matmul [2, 256] partials summed on host.
"""

import os

import numpy as np
import ml_dtypes

# ---- problem constants (hardcoded; kernel.py must be self-contained) ----
GENES = 20000
LEVEL = 10000
STEPS = 7
DEG = 16
B = 256
C = 2
NCORES = 8

PC_D = LEVEL // NCORES          # 1250 real dsts per core per level
PC_DP = 1280                    # padded dsts per core (10 tiles x 128)
SLOT = DEG
NTILE = PC_DP // 128            # 10 tiles of 128 dsts per core-level
NBLK = NTILE * 16               # 160 weight blocks per core-level
CH = 5                          # gather chunks per core-level (4096 idxs)
NIDX = PC_DP * SLOT // CH       # 4096 gather indices per chunk
IDXCOL = NIDX // 16             # 256 idx columns (16-partition wrap)
LVL_ROWS = NCORES * PC_DP       # 10240 rows per level table
SUB_MAX = int(os.environ.get("K2_SUB_MAX", "896"))  # idxs per dma_gather

BF16 = ml_dtypes.bfloat16

_COMPILED = {}
LAST_RESULT = None  # BassKernelResults of the most recent run (for test.py)


def _lvlrow(pos):
    """Level-k position (0..9999) -> row in the padded level table."""
    return PC_DP * (pos // PC_D) + pos % PC_D


def _prep(inputs):
    """Host-side: build per-core index/weight tables from runtime inputs."""
    X = np.asarray(inputs["X"], np.float32)
    ew = np.asarray(inputs["edge_weight"], np.float32)
    nb = np.asarray(inputs["node_bias"], np.float32)
    hW = np.asarray(inputs["head_W"], np.float32)
    hb = np.asarray(inputs["head_b"], np.float32)
    gm = np.asarray(inputs["gene_map"]).astype(np.int64)
    src = np.asarray(inputs["src"]).astype(np.int64)
    dpos = np.asarray(inputs["dst_pos"]).astype(np.int64)
    du = np.asarray(inputs["dst_unique"]).astype(np.int64)
    eid = np.asarray(inputs["eid"]).astype(np.int64)
    roots = np.asarray(inputs["root_ids"]).astype(np.int64)

    assert X.shape == (B, GENES) and src.shape == (STEPS, LEVEL * DEG)

    # X table: row g holds node gene_map[g]'s batch vector.
    x_tab = np.ascontiguousarray(X.T.astype(BF16))
    row_of_gene = np.empty(GENES, np.int64)
    row_of_gene[gm] = np.arange(GENES)

    max_id = int(du.max()) + 1
    idx_all = np.empty((NCORES, STEPS, CH, 16, IDXCOL), np.int16)
    w_all = np.empty((NCORES, 128, STEPS * NBLK, 64), BF16)
    bias_all = np.zeros((NCORES, 128, STEPS, NTILE), np.float32)

    lp = np.arange(PC_DP)
    valid = lp < PC_D
    kk = np.arange(128)

    for k in range(STEPS):
        order = np.argsort(dpos[k], kind="stable")
        assert np.all(np.bincount(dpos[k], minlength=LEVEL) == DEG), (
            "kernel assumes exact in-degree 16"
        )
        srcs = src[k][order]                 # [LEVEL*DEG] sorted by dst
        ws = ew[eid[k][order]]

        if k == 0:
            rows = row_of_gene[srcs]
        else:
            pos_of = np.full(max_id, -1, np.int64)
            pos_of[du[k - 1]] = np.arange(LEVEL)
            p = pos_of[srcs]
            assert (p >= 0).all(), "src outside previous level"
            rows = _lvlrow(p)

        rows = rows.reshape(LEVEL, DEG)
        ws = ws.reshape(LEVEL, DEG)
        for c in range(NCORES):
            pos_v = c * PC_D + lp[valid]  # real level positions for this core
            slot_idx = np.zeros((PC_DP, SLOT), np.int64)
            slot_w = np.zeros((PC_DP, SLOT), np.float32)
            slot_idx[valid] = rows[pos_v]
            slot_w[valid] = ws[pos_v]
            bias_all[c, :, k, :] = np.where(
                valid, nb[du[k][np.minimum(lp + c * PC_D, LEVEL - 1)]], 0.0
            ).reshape(NTILE, 128).T
            # gather idx: dst-major flat order, wrapped i -> [i%16, i//16]
            fl = slot_idx.reshape(CH, NIDX).astype(np.int16)
            idx_all[c, k] = fl.reshape(CH, IDXCOL, 16).transpose(0, 2, 1)
            # weight blocks: block cb (row range 128cb..+127), within its
            # 64-dst group (8 blocks) at position b=cb%8: row k -> col
            # 8b + k//16 carries the edge weight.
            wfl = slot_w.reshape(NBLK, 128)           # [block, row-in-block]
            wb = np.zeros((NBLK, 128, 64), np.float32)
            for b in range(8):
                wb[b::8, kk, 8 * b + kk // 16] = wfl[b::8, kk]
            w_all[c, :, k * NBLK:(k + 1) * NBLK, :] = (
                wb.transpose(1, 0, 2).astype(BF16)
            )

    # idx_tab per core: [128, STEPS*CH*IDXCOL], 16-row pattern tiled to 128
    idx16 = idx_all.reshape(NCORES, STEPS * CH, 16, IDXCOL)
    idx16 = idx16.transpose(0, 2, 1, 3).reshape(NCORES, 16, STEPS * CH * IDXCOL)
    idx_tab = np.tile(idx16, (1, NCORES, 1))  # [NCORES, 128, S*CH*IDXCOL]

    # head: W_eff[node] = sum of head_W rows whose root_ids hit that node
    W_eff = np.zeros((max_id, C), np.float32)
    np.add.at(W_eff, np.minimum(roots, max_id - 1), hW)
    head_tabs = []
    for c in range(NCORES):
        Wc = np.zeros((PC_DP, C), np.float32)
        Wc[valid] = W_eff[du[STEPS - 1][c * PC_D + lp[valid]]]
        head_tabs.append(
            np.ascontiguousarray(
                Wc.reshape(NTILE, 128, C).transpose(1, 0, 2)
            ).astype(BF16)
        )  # [128, 10, 2]

    has_bias = bool(np.any(nb != 0.0))
    in_maps = []
    for c in range(NCORES):
        m = {
            "x_tab": x_tab,
            "idx_tab": np.ascontiguousarray(idx_tab[c]),
            "w_tab": np.ascontiguousarray(w_all[c].reshape(128, -1)),
            "head_w": head_tabs[c],
        }
        if has_bias:
            m["bias_tab"] = np.ascontiguousarray(bias_all[c].reshape(128, -1))
        in_maps.append(m)
    return in_maps, hb, has_bias


def _build_nc(has_bias):
    import concourse.bacc as bacc
    import concourse.mybir as mybir
    import concourse.tile as tile

    f32 = mybir.dt.float32
    bf16 = mybir.dt.bfloat16
    i16 = mybir.dt.int16

    nc = bacc.Bacc(num_devices=NCORES)
    x_tab = nc.declare_dram_parameter("x_tab", [GENES, B], bf16, isOutput=False)
    idx_tab = nc.declare_dram_parameter(
        "idx_tab", [128, STEPS * CH * IDXCOL], i16, isOutput=False
    )
    w_tab = nc.declare_dram_parameter(
        "w_tab", [128, STEPS * NBLK * 64], bf16, isOutput=False
    )
    head_w = nc.declare_dram_parameter(
        "head_w", [128, NTILE, C], bf16, isOutput=False
    )
    bias_tab = None
    if has_bias:
        bias_tab = nc.declare_dram_parameter(
            "bias_tab", [128, STEPS * NTILE], f32, isOutput=False
        )
    out_partial = nc.declare_dram_parameter("out_partial", [C, B], f32, isOutput=True)

    # level tables (gather sources); collective output should be Shared
    lvl = [
        nc.dram_tensor(f"lvl{i}", [LVL_ROWS, B], bf16, addr_space="Shared")
        for i in range(2)
    ]
    own_slice = nc.dram_tensor("own_slice", [PC_DP, B], bf16)

    # sub-gather split: <=SUB_MAX idxs per instruction, multiples of 128
    subs = []
    off = 0
    while off < NIDX:
        ni = min(SUB_MAX, NIDX - off)
        subs.append((off, ni))
        off += ni

    with tile.TileContext(nc) as tc:
        with (
            tc.tile_pool(name="const", bufs=1) as const_pool,
            tc.tile_pool(name="msg", bufs=3) as msg_pool,
            tc.tile_pool(name="wlvl", bufs=2) as w_pool,
            tc.tile_pool(name="ps", bufs=4, space="PSUM") as psum_pool,
            tc.tile_pool(name="pshead", bufs=1, space="PSUM") as psum_head,
        ):
            # --- persistent setup ---
            idx_sb = const_pool.tile([128, STEPS * CH * IDXCOL], i16)
            nc.sync.dma_start(out=idx_sb[:], in_=idx_tab[:, :])
            hw_sb = const_pool.tile([128, NTILE, C], bf16)
            nc.sync.dma_start(out=hw_sb[:], in_=head_w[:, :, :])
            bias_sb = None
            if has_bias:
                bias_sb = const_pool.tile([128, STEPS, NTILE], f32)
                nc.sync.dma_start(
                    out=bias_sb[:],
                    in_=bias_tab[:, :].rearrange("p (k g) -> p k g", g=NTILE),
                )
            nm_sb = const_pool.tile([128, NTILE, B], bf16)
            sub_regs = {ni: nc.gpsimd.to_reg(ni) for ni in {s[1] for s in subs}}

            w_lvl = [None, None]
            for k in range(STEPS):
                srctab = x_tab if k == 0 else lvl[(k - 1) % 2]
                # stream this level's weight blocks (double-buffered)
                wt = w_pool.tile([128, NBLK, 64], bf16, tag="wl")
                w_lvl[k % 2] = wt
                nc.sync.dma_start(
                    out=w_lvl[k % 2][:],
                    in_=w_tab[
                        :, k * NBLK * 64:(k + 1) * NBLK * 64
                    ].rearrange("p (t m) -> p t m", m=64),
                )
                w_sb = w_lvl[k % 2]
                for u in range(CH):
                    r = k * CH + u
                    msg = msg_pool.tile([128, NIDX // 128, B], bf16, tag="msg")
                    for i0, ni in subs:
                        nc.gpsimd.dma_gather(
                            out_ap=msg[:, i0 // 128:(i0 + ni) // 128, :],
                            in_ap=srctab[:, :],
                            idxs_ap=idx_sb[
                                :,
                                r * IDXCOL + i0 // 16 : r * IDXCOL + (i0 + ni) // 16,
                            ],
                            num_idxs=ni,
                            num_idxs_reg=sub_regs[ni],
                            elem_size=B,
                            transpose=False,
                        )
                    for half in range(NIDX // 2048):
                        ps = psum_pool.tile([128, B], f32)
                        for grp in range(2):
                            for b in range(8):
                                blk = u * (NIDX // 128) + half * 16 + grp * 8 + b
                                nc.tensor.matmul(
                                    out=ps[64 * grp:64 * (grp + 1), :],
                                    lhsT=w_sb[:, blk, :],
                                    rhs=msg[:, half * 16 + grp * 8 + b, :],
                                    start=(b == 0),
                                    stop=(b == 7),
                                )
                        g = u * (NIDX // 2048) + half
                        if has_bias:
                            nc.scalar.activation(
                                out=nm_sb[:, g, :], in_=ps[:],
                                func=mybir.ActivationFunctionType.Tanh,
                                bias=bias_sb[:, k, g:g + 1],
                            )
                        else:
                            nc.scalar.activation(
                                out=nm_sb[:, g, :], in_=ps[:],
                                func=mybir.ActivationFunctionType.Tanh,
                            )
                if k < STEPS - 1:
                    # publish level k+1: own 1280 rows -> all-gather to lvl[k%2]
                    # own_slice row (g*128+p) <- nm_sb[p, g, :]
                    nc.sync.dma_start(
                        out=own_slice[:, :].rearrange("(g p) b -> p g b", p=128),
                        in_=nm_sb[:],
                    )
                    nc.gpsimd.collective_compute(
                        "AllGather",
                        mybir.AluOpType.bypass,
                        replica_groups=[list(range(NCORES))],
                        ins=[own_slice[:, :]],
                        outs=[lvl[k % 2][:, :]],
                    )
                else:
                    pm = psum_head.tile([C, B], f32)
                    for g in range(NTILE):
                        nc.tensor.matmul(
                            out=pm[:],
                            lhsT=hw_sb[:, g, :],
                            rhs=nm_sb[:, g, :],
                            start=(g == 0),
                            stop=(g == NTILE - 1),
                        )
                    res = const_pool.tile([C, B], f32)
                    nc.vector.tensor_copy(out=res[:], in_=pm[:])
                    nc.sync.dma_start(out=out_partial[:, :], in_=res[:])
    nc.finalize()
    return nc


def kernel(**inputs):
    global LAST_RESULT
    from concourse.bass_utils import run_bass_kernel_spmd

    in_maps, hb, has_bias = _prep(inputs)

    key = ("nc2", has_bias, SUB_MAX)
    if key not in _COMPILED:
        _COMPILED[key] = _build_nc(has_bias)
    nc = _COMPILED[key]

    trace = os.environ.get("BASS_TRACE", "0") == "1"
    if trace:
        try:
            import antenv.axon_hooks  # noqa: F401
        except ImportError:
            trace = False
    res = run_bass_kernel_spmd(
        nc, in_maps, core_ids=list(range(NCORES)), trace=trace
    )
    LAST_RESULT = res

    partials = np.stack(
        [np.asarray(r["out_partial"], np.float32) for r in res.results]
    )
    out = partials.sum(axis=0).T + hb[None, :]
    return out.astype(np.float32)


# revision 3
# speedup vs baseline: 1.1355x; 1.0196x over previous
"""DAG-BiNN message passing on 8 TRN2 NeuronCores — v3 hybrid.

The per-edge dma_gather costs ~7.9 ns/descriptor of GpSimd(Q7) descriptor
generation — the hard bottleneck of any pure-gather kernel (143k descriptors
per core = 1.13 ms). v3 splits each level's 1280 destinations per core:

- tiles 0..G-1 (128 dsts each): gather path (v2): dma_gather(transpose=False)
  pulls the 16 src rows per dst in dst-major order; PE applies weights and
  reduces via [128, 64]-weight-block matmuls into node-major psum; ACT tanh.
- tiles G..9: DENSE path — zero descriptors. agg[b, j] = sum_r lvl[r, b] *
  S[r, j] with S the host-built (10240 x 768) scatter matrix (mostly zeros).
  Per 128-row block: lhsT = level block [128 rows, 128 batch-half] (SBUF-
  resident copy of the level), rhs = S block [128 rows, 768] streamed from
  DRAM, accumulated into [128 b, 512/256] psum over all 80 blocks; tanh on
  ACT, then PE-transpose back to node-major.

Level 0 (srcs = 20000 gene rows) is pure gather path (G=10). Publish between
levels: own-slice DMA + AllGather collective, as v1/v2.
"""

import os

import numpy as np
import ml_dtypes

GENES = 20000
LEVEL = 10000
STEPS = 7
DEG = 16
B = 256
C = 2
NCORES = 8

PC_D = LEVEL // NCORES          # 1250 real dsts per core per level
PC_DP = 1280                    # padded dsts per core (10 tiles x 128)
SLOT = DEG
NTILE = PC_DP // 128            # 10 tiles of 128 dsts per core-level
NIDX = 4096                     # gather indices per chunk (2 tiles)
IDXCOL = NIDX // 16
LVL_ROWS = NCORES * PC_DP       # 10240 rows per level table
NRB = LVL_ROWS // 128           # 80 row-blocks per level table
SUB_MAX = int(os.environ.get("K3_SUB_MAX", "896"))

# gather tiles per level (rest go dense)
G_LIST = [int(x) for x in os.environ.get(
    "K3_G", "8,4,4,4,4,4,4").split(",")]
assert len(G_LIST) == STEPS
assert all(g % 2 == 0 for g in G_LIST) and all(0 <= g <= 10 for g in G_LIST)
XRB = (GENES + 127) // 128          # 157 x-table row blocks
XPAD = XRB * 128                    # 20096 padded x rows

BF16 = ml_dtypes.bfloat16

_COMPILED = {}
LAST_RESULT = None


def _lvlrow(pos):
    """Level position -> S-matrix row (block-major: c*1280 + g*128 + p)."""
    return PC_DP * (pos // PC_D) + pos % PC_D


def _lvlq(pos):
    """Level position -> DRAM table row (p-major within a core slice:
    c*1280 + p*10 + g) so per-partition slice reads are contiguous."""
    c, l = pos // PC_D, pos % PC_D
    return c * PC_DP + (l % 128) * NTILE + l // 128


def _xq(r):
    """Logical x row -> p-major x table row (q = p*XRB + rb)."""
    return (r % 128) * XRB + r // 128


def _dense_segs(g):
    d = (NTILE - g) * 128
    segs = []
    off = 0
    while off < d:
        s = min(512, d - off)
        segs.append((off, s))
        off += s
    return segs


def _prep(inputs):
    X = np.asarray(inputs["X"], np.float32)
    ew = np.asarray(inputs["edge_weight"], np.float32)
    nb = np.asarray(inputs["node_bias"], np.float32)
    hW = np.asarray(inputs["head_W"], np.float32)
    hb = np.asarray(inputs["head_b"], np.float32)
    gm = np.asarray(inputs["gene_map"]).astype(np.int64)
    src = np.asarray(inputs["src"]).astype(np.int64)
    dpos = np.asarray(inputs["dst_pos"]).astype(np.int64)
    du = np.asarray(inputs["dst_unique"]).astype(np.int64)
    eid = np.asarray(inputs["eid"]).astype(np.int64)
    roots = np.asarray(inputs["root_ids"]).astype(np.int64)

    assert X.shape == (B, GENES) and src.shape == (STEPS, LEVEL * DEG)
    has_bias = bool(np.any(nb != 0.0))

    x_flat = np.zeros((XPAD, B), np.float32)
    x_flat[:GENES] = X.T
    x_tab = np.zeros((XPAD, B), np.float32)
    x_tab[_xq(np.arange(XPAD))] = x_flat   # p-major layout
    x_tab = np.ascontiguousarray(x_tab.astype(BF16))
    row_of_gene = np.empty(GENES, np.int64)
    row_of_gene[gm] = np.arange(GENES)

    max_id = int(du.max()) + 1
    n_chunks = [g // 2 for g in G_LIST]
    tot_chunks = sum(n_chunks)
    tot_gblk = sum(g * 16 for g in G_LIST)
    idx_all = np.zeros((NCORES, tot_chunks, 16, IDXCOL), np.int16)
    w_all = np.zeros((NCORES, 128, tot_gblk, 64), BF16)
    sd_all = [
        np.zeros(
            (NCORES, XPAD if k == 0 else LVL_ROWS,
             (NTILE - G_LIST[k]) * 128),
            np.float32,
        )
        for k in range(STEPS)
    ]

    lp = np.arange(PC_DP)
    valid = lp < PC_D
    kk = np.arange(128)

    chunk_off = 0
    gblk_off = 0
    for k in range(STEPS):
        g_k = G_LIST[k]
        order = np.argsort(dpos[k], kind="stable")
        assert np.all(np.bincount(dpos[k], minlength=LEVEL) == DEG)
        srcs = src[k][order]
        ws = ew[eid[k][order]]

        if k == 0:
            lrows = row_of_gene[srcs]
            rows_q = _xq(lrows)       # gather idx basis (p-major table)
            rows_s = lrows            # S row basis (block-major)
        else:
            pos_of = np.full(max_id, -1, np.int64)
            pos_of[du[k - 1]] = np.arange(LEVEL)
            p = pos_of[srcs]
            assert (p >= 0).all()
            rows_q = _lvlq(p)
            rows_s = _lvlrow(p)

        rows_q = rows_q.reshape(LEVEL, DEG)
        rows_s = rows_s.reshape(LEVEL, DEG)
        ws = ws.reshape(LEVEL, DEG)
        for c in range(NCORES):
            pos_v = c * PC_D + lp[valid]
            slot_idx = np.zeros((PC_DP, SLOT), np.int64)
            slot_s = np.zeros((PC_DP, SLOT), np.int64)
            slot_w = np.zeros((PC_DP, SLOT), np.float32)
            slot_idx[valid] = rows_q[pos_v]
            slot_s[valid] = rows_s[pos_v]
            slot_w[valid] = ws[pos_v]
            ngd = g_k * 128  # gather-path dst count
            # --- gather tables for tiles < g_k ---
            if ngd:
                fl = slot_idx[:ngd].reshape(g_k // 2, NIDX).astype(np.int16)
                idx_all[c, chunk_off:chunk_off + g_k // 2] = (
                    fl.reshape(-1, IDXCOL, 16).transpose(0, 2, 1)
                )
                nblk = g_k * 16
                wfl = slot_w[:ngd].reshape(nblk, 128)
                wb = np.zeros((nblk, 128, 64), np.float32)
                for b in range(8):
                    wb[b::8, kk, 8 * b + kk // 16] = wfl[b::8, kk]
                w_all[c, :, gblk_off:gblk_off + nblk, :] = (
                    wb.transpose(1, 0, 2).astype(BF16)
                )
            # --- dense S for tiles >= g_k (rows in block-major basis) ---
            if g_k < NTILE:
                r_arr = slot_s[ngd:].ravel()
                w_arr = slot_w[ngd:].ravel()
                c_arr = np.repeat(np.arange(PC_DP - ngd), SLOT)
                np.add.at(sd_all[k][c], (r_arr, c_arr), w_arr)
        chunk_off += g_k // 2
        gblk_off += g_k * 16

    idx16 = idx_all.reshape(NCORES, tot_chunks, 16, IDXCOL)
    idx16 = idx16.transpose(0, 2, 1, 3).reshape(NCORES, 16, tot_chunks * IDXCOL)
    idx_tab = np.tile(idx16, (1, NCORES, 1))

    W_eff = np.zeros((max_id, C), np.float32)
    np.add.at(W_eff, np.minimum(roots, max_id - 1), hW)
    head_tabs = []
    for c in range(NCORES):
        Wc = np.zeros((PC_DP, C), np.float32)
        Wc[valid] = W_eff[du[STEPS - 1][c * PC_D + lp[valid]]]
        head_tabs.append(
            np.ascontiguousarray(
                Wc.reshape(NTILE, 128, C).transpose(1, 0, 2)
            ).astype(BF16)
        )

    dense_levels = [k for k in range(STEPS) if G_LIST[k] < NTILE]
    sd_cols = max(((NTILE - G_LIST[k]) * 128 for k in dense_levels),
                  default=0)
    in_maps = []
    for c in range(NCORES):
        m = {
            "x_tab": x_tab,
            "idx_tab": np.ascontiguousarray(idx_tab[c]),
            "w_tab": np.ascontiguousarray(w_all[c].reshape(128, -1)),
            "head_w": head_tabs[c],
        }
        if sd_cols:
            pieces = []
            for k in dense_levels:
                s = sd_all[k][c]
                if s.shape[1] < sd_cols:
                    s = np.pad(s, ((0, 0), (0, sd_cols - s.shape[1])))
                pieces.append(s)
            m["sd_tab"] = np.ascontiguousarray(
                np.concatenate(pieces, axis=0).astype(BF16))
        in_maps.append(m)
    return in_maps, hb, has_bias


def _build_nc(has_bias):
    import concourse.bacc as bacc
    import concourse.mybir as mybir
    import concourse.tile as tile
    from concourse.masks import make_identity

    f32 = mybir.dt.float32
    bf16 = mybir.dt.bfloat16
    i16 = mybir.dt.int16
    Tanh = mybir.ActivationFunctionType.Tanh

    assert not has_bias, "v3 dense path has no bias support (inputs are zeros)"

    n_chunks = [g // 2 for g in G_LIST]
    tot_chunks = sum(n_chunks)
    tot_gblk = sum(g * 16 for g in G_LIST)
    dense_levels = [k for k in range(STEPS) if G_LIST[k] < NTILE]
    sd_rows = sum(XPAD if k == 0 else LVL_ROWS for k in dense_levels)
    sd_cols = max(((NTILE - G_LIST[k]) * 128 for k in dense_levels),
                  default=0)

    nc = bacc.Bacc(num_devices=NCORES)
    x_tab = nc.declare_dram_parameter("x_tab", [XPAD, B], bf16, isOutput=False)
    idx_tab = nc.declare_dram_parameter(
        "idx_tab", [128, tot_chunks * IDXCOL], i16, isOutput=False
    )
    w_tab = nc.declare_dram_parameter(
        "w_tab", [128, tot_gblk * 64], bf16, isOutput=False
    )
    head_w = nc.declare_dram_parameter(
        "head_w", [128, NTILE, C], bf16, isOutput=False
    )
    sd_tab = None
    if dense_levels:
        sd_tab = nc.declare_dram_parameter(
            "sd_tab", [sd_rows, sd_cols], bf16, isOutput=False
        )
    out_partial = nc.declare_dram_parameter("out_partial", [C, B], f32, isOutput=True)

    lvl = [
        nc.dram_tensor(f"lvl{i}", [LVL_ROWS, B], bf16, addr_space="Shared")
        for i in range(2)
    ]
    own_slice = nc.dram_tensor("own_slice", [PC_DP, B], bf16)

    subs = []
    off = 0
    while off < NIDX:
        ni = min(SUB_MAX, NIDX - off)
        subs.append((off, ni))
        off += ni

    SSEG = 20  # S row-blocks per streamed piece

    with tile.TileContext(nc) as tc:
        with (
            tc.tile_pool(name="const", bufs=1) as const_pool,
            tc.tile_pool(name="msg", bufs=2) as msg_pool,
            tc.tile_pool(name="wlvl", bufs=1) as w_pool,
            tc.tile_pool(name="lvres", bufs=1) as lv_pool,
            tc.tile_pool(name="xbp", bufs=2) as xb_pool,
            tc.tile_pool(name="sd", bufs=2) as sd_pool,
            tc.tile_pool(name="tmp", bufs=2) as tmp_pool,
            tc.tile_pool(name="ps", bufs=2, space="PSUM") as psum_pool,
            tc.tile_pool(name="psd", bufs=1, space="PSUM") as psd_pool,
            tc.tile_pool(name="pst", bufs=1, space="PSUM") as pst_pool,
        ):
            idx_sb = const_pool.tile([128, tot_chunks * IDXCOL], i16)
            nc.sync.dma_start(out=idx_sb[:], in_=idx_tab[:, :])
            hw_sb = const_pool.tile([128, NTILE, C], bf16)
            nc.sync.dma_start(out=hw_sb[:], in_=head_w[:, :, :])
            ident = const_pool.tile([128, 128], bf16)
            make_identity(nc, ident[:])
            nm_sb = const_pool.tile([128, NTILE, B], bf16)
            sub_regs = {ni: nc.gpsimd.to_reg(ni) for ni in {s[1] for s in subs}}

            chunk_off = 0
            gblk_off = 0
            sd_off = 0
            for k in range(STEPS):
                g_k = G_LIST[k]
                srctab = x_tab if k == 0 else lvl[(k - 1) % 2]
                nck = n_chunks[k]
                nblk = g_k * 16
                nrb_k = XRB if k == 0 else NRB

                # ---- stream this level's gather weight blocks ----
                if nblk:
                    wt = w_pool.tile([128, 160, 64], bf16, tag="wl")
                    nc.sync.dma_start(
                        out=wt[:, :nblk, :],
                        in_=w_tab[
                            :, gblk_off * 64:(gblk_off + nblk) * 64
                        ].rearrange("p (t m) -> p t m", m=64),
                    )

                # ---- dense path setup ----
                dense = k in dense_levels
                if dense:
                    segs = _dense_segs(g_k)
                    dcols = (NTILE - g_k) * 128
                    if k > 0:
                        # SBUF-resident copy of the level (lhsT source);
                        # p-major slices make each partition's read contiguous
                        lvr = lv_pool.tile([128, NRB, B], bf16, tag="lvr")
                        for c in range(NCORES):
                            nc.sync.dma_start(
                                out=lvr[:, c * NTILE:(c + 1) * NTILE, :],
                                in_=lvl[(k - 1) % 2][
                                    c * PC_DP:(c + 1) * PC_DP, :
                                ].rearrange("(p g) b -> p g b", g=NTILE),
                            )
                    psd = [[], []]
                    for bh in range(2):
                        for si, (o, s) in enumerate(segs):
                            pd = psd_pool.tile(
                                [128, s], f32, tag=f"psd{si}_{bh}")
                            psd[bh].append(pd)
                    # S stream piece boundaries
                    sseg_bounds = []
                    o = 0
                    while o < nrb_k:
                        n = min(SSEG, nrb_k - o)
                        sseg_bounds.append((o, n))
                        o += n
                else:
                    sseg_bounds = []

                def emit_sseg(sb):
                    o0, n = sb
                    sdb = sd_pool.tile([128, SSEG, sd_cols], bf16, tag="sdb")
                    nc.sync.dma_start(
                        out=sdb[:, :n, :dcols],
                        in_=sd_tab[
                            sd_off + o0 * 128:sd_off + (o0 + n) * 128, :dcols
                        ].rearrange("(rb p) m -> p rb m", p=128),
                    )
                    if k == 0:
                        # stream x lhsT blocks (p-major table: contiguous)
                        xb = xb_pool.tile([128, SSEG, B], bf16, tag="xb")
                        nc.sync.dma_start(
                            out=xb[:, :n, :],
                            in_=x_tab[:, :].rearrange(
                                "(p rb) b -> p rb b", rb=XRB)[:, o0:o0 + n, :],
                        )
                        lhsrc = xb
                        loff = o0
                    else:
                        lhsrc = lvr
                        loff = 0
                    for rl in range(n):
                        rb = o0 + rl
                        lsl = rb - loff if k == 0 else rb
                        for bh in range(2):
                            for si, (o, s) in enumerate(segs):
                                nc.tensor.matmul(
                                    out=psd[bh][si][:],
                                    lhsT=lhsrc[:, lsl, bh * 128:(bh + 1) * 128],
                                    rhs=sdb[:, rl, o:o + s],
                                    start=(rb == 0),
                                    stop=(rb == nrb_k - 1),
                                )

                def emit_gather_dma(u):
                    r = chunk_off + u
                    msg = msg_pool.tile([128, NIDX // 128, B], bf16, tag="msg")
                    for i0, ni in subs:
                        nc.gpsimd.dma_gather(
                            out_ap=msg[:, i0 // 128:(i0 + ni) // 128, :],
                            in_ap=srctab[:, :],
                            idxs_ap=idx_sb[
                                :,
                                r * IDXCOL + i0 // 16:r * IDXCOL + (i0 + ni) // 16,
                            ],
                            num_idxs=ni,
                            num_idxs_reg=sub_regs[ni],
                            elem_size=B,
                            transpose=False,
                        )
                    return msg

                def emit_gather_compute(u, msg):
                    for half in range(NIDX // 2048):
                        ps = psum_pool.tile([128, B], f32)
                        for grp in range(2):
                            for b in range(8):
                                blk = u * 32 + half * 16 + grp * 8 + b
                                nc.tensor.matmul(
                                    out=ps[64 * grp:64 * (grp + 1), :],
                                    lhsT=wt[:, blk, :],
                                    rhs=msg[:, half * 16 + grp * 8 + b, :],
                                    start=(b == 0),
                                    stop=(b == 7),
                                )
                        g = u * 2 + half
                        nc.scalar.activation(
                            out=nm_sb[:, g, :], in_=ps[:], func=Tanh)

                # ---- interleaved emission: keep Q7 fed while PE runs ----
                msgs = {}
                for u in range(min(2, nck)):
                    msgs[u] = emit_gather_dma(u)
                nstep = max(len(sseg_bounds), nck)
                for i in range(nstep):
                    if i < len(sseg_bounds):
                        emit_sseg(sseg_bounds[i])
                    if i < nck:
                        emit_gather_compute(i, msgs.pop(i))
                        if i + 2 < nck:
                            msgs[i + 2] = emit_gather_dma(i + 2)
                if dense:
                    sd_off += nrb_k * 128

                # ---- dense path finish: tanh + transpose to node-major ----
                if dense:
                    for bh in range(2):
                        tmps = []
                        for si, (o, s) in enumerate(segs):
                            tt = tmp_pool.tile([128, 512], bf16, tag=f"tt{si}")
                            nc.scalar.activation(
                                out=tt[:, :s], in_=psd[bh][si][:], func=Tanh)
                            tmps.append(tt)
                        for t in range(NTILE - g_k):
                            o = t * 128
                            si = o // 512
                            oo = o % 512
                            pt = pst_pool.tile([128, 128], bf16)
                            nc.tensor.transpose(
                                out=pt[:],
                                in_=tmps[si][:, oo:oo + 128],
                                identity=ident[:],
                            )
                            nc.scalar.copy(
                                out=nm_sb[:, g_k + t, bh * 128:(bh + 1) * 128],
                                in_=pt[:],
                            )

                chunk_off += nck
                gblk_off += nblk

                if k < STEPS - 1:
                    # publish own slice in p-major order (contiguous rows)
                    nc.sync.dma_start(
                        out=own_slice[:, :].rearrange("(p g) b -> p g b", g=NTILE),
                        in_=nm_sb[:],
                    )
                    nc.gpsimd.collective_compute(
                        "AllGather",
                        mybir.AluOpType.bypass,
                        replica_groups=[list(range(NCORES))],
                        ins=[own_slice[:, :]],
                        outs=[lvl[k % 2][:, :]],
                    )
                else:
                    pm = psd_pool.tile([C, B], f32, tag="pm")
                    for g in range(NTILE):
                        nc.tensor.matmul(
                            out=pm[:],
                            lhsT=hw_sb[:, g, :],
                            rhs=nm_sb[:, g, :],
                            start=(g == 0),
                            stop=(g == NTILE - 1),
                        )
                    res = const_pool.tile([C, B], f32)
                    nc.vector.tensor_copy(out=res[:], in_=pm[:])
                    nc.sync.dma_start(out=out_partial[:, :], in_=res[:])
    nc.finalize()
    return nc


def kernel(**inputs):
    global LAST_RESULT
    from concourse.bass_utils import run_bass_kernel_spmd

    in_maps, hb, has_bias = _prep(inputs)

    key = ("nc3", has_bias, SUB_MAX, tuple(G_LIST))
    if key not in _COMPILED:
        _COMPILED[key] = _build_nc(has_bias)
    nc = _COMPILED[key]

    trace = os.environ.get("BASS_TRACE", "0") == "1"
    if trace:
        try:
            import antenv.axon_hooks  # noqa: F401
        except ImportError:
            trace = False
    res = run_bass_kernel_spmd(
        nc, in_maps, core_ids=list(range(NCORES)), trace=trace
    )
    LAST_RESULT = res

    partials = np.stack(
        [np.asarray(r["out_partial"], np.float32) for r in res.results]
    )
    out = partials.sum(axis=0).T + hb[None, :]
    return out.astype(np.float32)
